# revision 1
# baseline (speedup 1.0000x reference)
"""MoE FFN (shared SwiGLU + 8 dense-routed SwiGLU experts) on 8 TRN2 NeuronCores.

Strategy: data-parallel over batch (B=16 -> 2 batches/core), expert weights
replicated. All activations kept feature-major ([feature, token]) so every
matmul consumes natural-layout weights and host-pre-transposed x with zero
on-chip transposes. Matmul operands are bf16 (FWL weight loads hide under the
N=512 moving stream); accumulation is fp32 in PSUM. The routed mixture weight
and all down-projection biases are folded in for free (rw into the up-branch
elementwise op, biases into the first unit's PSUM->SBUF accumulate).
"""
import sys

if "/opt/trn_rl_repo" not in sys.path:
    sys.path.insert(0, "/opt/trn_rl_repo")

import numpy as np
import ml_dtypes

import concourse.bass as bass  # noqa: F401  (registers engine classes)
import concourse.tile as tile
from concourse import bacc, mybir
from concourse import bass_utils

F32 = mybir.dt.float32
BF16 = mybir.dt.bfloat16
Silu = mybir.ActivationFunctionType.Silu
Alu = mybir.AluOpType

B, K, D = 16, 1024, 512
HS, HR, E = 2048, 1024, 8
NCORES = 8
BL = B // NCORES          # batches per core = 2
T = BL * K                # tokens per core = 2048
TT = 512                  # token tile (matmul moving dim)
NTT = T // TT             # 4 token tiles per core
NU = 2 + E                # units: 2 shared halves + 8 experts
HU = 1024                 # hidden width of every unit
NH = HU // 128            # 8 h-tiles per unit
ND = D // 128             # 4 d-tiles
NK = D // 128             # 4 contraction tiles for gate/up


def _build():
    nc = bacc.Bacc("TRN2", target_bir_lowering=False, debug=False,
                   num_devices=NCORES)
    xT = nc.dram_tensor("xT", (D, T), BF16, kind="ExternalInput")
    wg = nc.dram_tensor("wg", (NU, D, HU), BF16, kind="ExternalInput")
    wu = nc.dram_tensor("wu", (NU, D, HU), BF16, kind="ExternalInput")
    wd = nc.dram_tensor("wd", (NU, HU, D), BF16, kind="ExternalInput")
    gb = nc.dram_tensor("gb", (128, NU, NH), F32, kind="ExternalInput")
    ub = nc.dram_tensor("ub", (128, NU, NH), F32, kind="ExternalInput")
    rw = nc.dram_tensor("rw", (128, NU, NTT), F32, kind="ExternalInput")
    cv = nc.dram_tensor("cv", (128, ND, NTT), F32, kind="ExternalInput")
    outT = nc.dram_tensor("outT", (D, T), F32, kind="ExternalOutput")

    with tile.TileContext(nc) as tc:
        with (
            tc.tile_pool(name="persist", bufs=1) as persist,
            tc.tile_pool(name="wpool", bufs=2) as wpool,
            tc.tile_pool(name="hpool", bufs=2) as hpool,
            tc.tile_pool(name="spool", bufs=4) as spool,
            tc.tile_pool(name="gups", bufs=2, space="PSUM") as gups,
            tc.tile_pool(name="ops", bufs=1, space="PSUM") as opsp,
        ):
            xt = persist.tile([128, NK, T], BF16)
            oacc = persist.tile([128, ND, T], F32)
            gbt = persist.tile([128, NU, NH], F32)
            ubt = persist.tile([128, NU, NH], F32)
            rwt = persist.tile([128, NU, NTT], F32)
            cvt = persist.tile([128, ND, NTT], F32)

            def load_unit_weights(u, wgt, wut, wdt, first=False):
                # split gate/up weight loads by h-half so the first matmuls of
                # a unit only wait on the slices they actually read; for the
                # first unit, interleave with the x loads the same matmuls
                # need so the critical descriptors spread across DMA queues.
                for half in range(2):
                    hh = slice(half * 512, (half + 1) * 512)
                    for k in range(NK):
                        nc.gpsimd.dma_start(wgt[:, k, hh],
                                          wg.ap()[u, k * 128:(k + 1) * 128, hh])
                        if first:
                            nc.sync.dma_start(
                                xt[:, k, slice(half * TT, (half + 1) * TT)],
                                xT.ap()[k * 128:(k + 1) * 128,
                                        half * TT:(half + 1) * TT])
                    if first and half == 0:
                        nc.sync.dma_start(gbt[:], gb.ap()[:])
                        nc.sync.dma_start(ubt[:], ub.ap()[:])
                        nc.sync.dma_start(rwt[:], rw.ap()[:])
                        nc.sync.dma_start(cvt[:], cv.ap()[:])
                    for k in range(NK):
                        nc.gpsimd.dma_start(wut[:, k, hh],
                                          wu.ap()[u, k * 128:(k + 1) * 128, hh])
                for k in range(NH):
                    nc.gpsimd.dma_start(wdt[:, k, :], wd.ap()[u, k * 128:(k + 1) * 128, :])

            w0 = (wpool.tile([128, NK, HU], BF16, tag="wg", name="wgt0"),
                  wpool.tile([128, NK, HU], BF16, tag="wu", name="wut0"),
                  wpool.tile([128, NH, D], BF16, tag="wd", name="wdt0"))
            load_unit_weights(0, *w0, first=True)
            # remaining x token tiles
            for t in range(2, NTT):
                tok = slice(t * TT, (t + 1) * TT)
                for k in range(NK):
                    nc.sync.dma_start(xt[:, k, tok],
                                      xT.ap()[k * 128:(k + 1) * 128, tok])

            for u in range(NU):
                if u == 0:
                    wgt, wut, wdt = w0
                else:
                    wgt = wpool.tile([128, NK, HU], BF16, tag="wg")
                    wut = wpool.tile([128, NK, HU], BF16, tag="wu")
                    wdt = wpool.tile([128, NH, D], BF16, tag="wd")
                    load_unit_weights(u, wgt, wut, wdt)

                for t in range(NTT):
                    tok = slice(t * TT, (t + 1) * TT)
                    hts = [hpool.tile([128, TT], BF16, tag=f"h{hi}",
                                      name=f"h{hi}_u{u}t{t}") for hi in range(NH)]
                    for hi in range(NH):
                        hc = slice(hi * 128, (hi + 1) * 128)
                        gps = gups.tile([128, TT], F32, tag="g")
                        for k in range(NK):
                            nc.tensor.matmul(gps[:], wgt[:, k, hc], xt[:, k, tok],
                                             start=(k == 0), stop=(k == NK - 1))
                        ups = gups.tile([128, TT], F32, tag="u")
                        for k in range(NK):
                            nc.tensor.matmul(ups[:], wut[:, k, hc], xt[:, k, tok],
                                             start=(k == 0), stop=(k == NK - 1))
                        sg = spool.tile([128, TT], F32, tag="sg")
                        nc.scalar.activation(sg[:], gps[:], Silu,
                                             bias=gbt[:, u, hi:hi + 1])
                        su = spool.tile([128, TT], F32, tag="su")
                        nc.vector.tensor_scalar(su[:], ups[:],
                                                ubt[:, u, hi:hi + 1],
                                                rwt[:, u, t:t + 1],
                                                Alu.add, Alu.mult)
                        nc.vector.tensor_tensor(hts[hi][:], sg[:], su[:], Alu.mult)

                    odst = [opsp.tile([128, TT], F32, tag=f"o{di}",
                                      name=f"o{di}_u{u}t{t}") for di in range(ND)]
                    for k in range(NH):
                        for di in range(ND):
                            nc.tensor.matmul(odst[di][:],
                                             wdt[:, k, di * 128:(di + 1) * 128],
                                             hts[k][:],
                                             start=(k == 0), stop=(k == NH - 1),
                                             skip_group_check=True)
                    for di in range(ND):
                        dst = oacc[:, di, tok]
                        if u == 0:
                            nc.vector.tensor_scalar_add(dst, odst[di][:],
                                                        cvt[:, di, t:t + 1])
                        else:
                            nc.vector.tensor_tensor(dst, odst[di][:], dst, Alu.add)
                        if u == NU - 1:
                            nc.sync.dma_start(
                                outT.ap()[di * 128:(di + 1) * 128, tok], dst)
    nc.compile()
    return nc


_NC = None


def _get_nc():
    global _NC
    if _NC is None:
        _NC = _build()
    return _NC


def _bf16(a):
    return np.ascontiguousarray(np.asarray(a, np.float32)).astype(ml_dtypes.bfloat16)


def _pack_shared(Ws_gate, bs_gate, Ws_up, bs_up, Ws_down, bs_down,
                 Wr_gate, br_gate, Wr_up, br_up, Wr_down, br_down):
    wg = np.empty((NU, D, HU), np.float32)
    wu = np.empty((NU, D, HU), np.float32)
    wd = np.empty((NU, HU, D), np.float32)
    gb = np.empty((128, NU, NH), np.float32)
    ub = np.empty((128, NU, NH), np.float32)
    for u in range(2):
        h0 = slice(u * HU, (u + 1) * HU)
        wg[u] = Ws_gate[:, h0]
        wu[u] = Ws_up[:, h0]
        wd[u] = Ws_down[h0, :]
        gb[:, u, :] = np.asarray(bs_gate[h0]).reshape(NH, 128).T
        ub[:, u, :] = np.asarray(bs_up[h0]).reshape(NH, 128).T
    for e in range(E):
        wg[2 + e] = Wr_gate[e]
        wu[2 + e] = Wr_up[e]
        wd[2 + e] = Wr_down[e]
        gb[:, 2 + e, :] = np.asarray(br_gate[e]).reshape(NH, 128).T
        ub[:, 2 + e, :] = np.asarray(br_up[e]).reshape(NH, 128).T
    return (_bf16(wg), _bf16(wu), _bf16(wd),
            np.ascontiguousarray(gb), np.ascontiguousarray(ub))


def _run(inputs, trace=False):
    x = np.asarray(inputs["x"], np.float32)
    rweights = np.asarray(inputs["routing_weights"], np.float32)
    wg, wu, wd, gb, ub = _pack_shared(
        np.asarray(inputs["Ws_gate"], np.float32), inputs["bs_gate"],
        np.asarray(inputs["Ws_up"], np.float32), inputs["bs_up"],
        np.asarray(inputs["Ws_down"], np.float32), inputs["bs_down"],
        np.asarray(inputs["Wr_gate"], np.float32), inputs["br_gate"],
        np.asarray(inputs["Wr_up"], np.float32), inputs["br_up"],
        np.asarray(inputs["Wr_down"], np.float32), inputs["br_down"])
    bs_down = np.asarray(inputs["bs_down"], np.float32)
    br_down = np.asarray(inputs["br_down"], np.float32)
    # down-bias vector per batch: bs_down + sum_e rw[b,e]*br_down[e]
    cfull = bs_down[None, :] + rweights @ br_down       # [B, D]

    in_maps = []
    for i in range(NCORES):
        xT = _bf16(x[BL * i:BL * (i + 1)].reshape(T, D).T)
        rw = np.ones((128, NU, NTT), np.float32)
        cv = np.empty((128, ND, NTT), np.float32)
        for t in range(NTT):
            bg = BL * i + t // (K // TT)
            for e in range(E):
                rw[:, 2 + e, t] = rweights[bg, e]
            cv[:, :, t] = cfull[bg].reshape(ND, 128).T
        in_maps.append({"xT": np.ascontiguousarray(xT), "wg": wg, "wu": wu,
                        "wd": wd, "gb": gb, "ub": ub,
                        "rw": np.ascontiguousarray(rw),
                        "cv": np.ascontiguousarray(cv)})

    res = bass_utils.run_bass_kernel_spmd(_get_nc(), in_maps,
                                          core_ids=list(range(NCORES)),
                                          trace=trace)
    out = np.empty((B, K, D), np.float32)
    for i in range(NCORES):
        out[BL * i:BL * (i + 1)] = res.results[i]["outT"].T.reshape(BL, K, D)
    return out, res


def kernel(**inputs) -> np.ndarray:
    out, _ = _run(inputs, trace=False)
    return out



# revision 5
# speedup vs baseline: 1.2801x; 1.2801x over previous
"""MoE FFN (shared SwiGLU + 8 dense-routed SwiGLU experts) on 8 TRN2 NeuronCores.

Strategy: data-parallel over batch (B=16 -> 2 batches/core). The 10 uniform
512->1024->512 SwiGLU units (2 shared halves + 8 experts) run with per-unit
precision: shared units in bf16, expert units in fp8e4m3 DoubleRow matmuls
(2x PE throughput). Expert quantization errors are independent across the 8
experts and average down under the routing weights, keeping total rel err
~1.8e-2 (<2e-2 gate) while cutting PE time to ~0.6x of the bf16 roofline.

Loop order is token-tile-outer / unit-inner so all 10 units' down-projections
accumulate directly in PSUM at a common scale (shared h pre-scaled by
SW*C=32768 in bf16, which is exact in floating point). One drain per
(d-tile, token-tile) applies the output bias and descale in a single
vector op. Weights stream per (t,u) double-buffered; DMA (~57MB) hides
under the ~500us of PE work.
"""
import sys

if "/opt/trn_rl_repo" not in sys.path:
    sys.path.insert(0, "/opt/trn_rl_repo")

import numpy as np
import ml_dtypes

import concourse.bass as bass  # noqa: F401  (registers engine classes)
import concourse.tile as tile
from concourse import bacc, mybir
from concourse import bass_utils

F32 = mybir.dt.float32
BF16 = mybir.dt.bfloat16
FP8 = mybir.dt.float8e4
Silu = mybir.ActivationFunctionType.Silu
ACT = Silu  # debug harnesses may swap to Sigmoid (CoreSim lacks Silu)
Alu = mybir.AluOpType
DR = mybir.MatmulPerfMode.DoubleRow

B, K, D = 16, 1024, 512
HS, HR, E = 2048, 1024, 8
NCORES = 8
BL = B // NCORES          # batches per core = 2
T = BL * K                # tokens per core = 2048
TT = 512                  # token tile (matmul moving dim)
NTT = T // TT             # 4 token tiles per core
NU = 2 + E                # units: 2 shared halves + 8 experts
HU = 1024                 # hidden width of every unit
NH = HU // 128            # 8 h-tiles per unit
ND = D // 128             # 4 d-tiles
NK = D // 128             # 4 contraction tiles for gate/up

SX = 32.0                 # fp8 x scale
SWQ = 1024.0              # fp8 weight scale
C = 32.0                  # h-domain scale for expert fp8 h
SXW = SX * SWQ            # expert gate/up psum scale
SDC = SWQ * C             # common down psum scale (all units)

# per-unit precision: shared halves bf16, experts fp8
UNIT_FP8 = [False, False] + [True] * E


def _build():
    nc = bacc.Bacc("TRN2", target_bir_lowering=False, debug=False,
                   num_devices=NCORES)
    xTb = nc.dram_tensor("xTb", (D, T), BF16, kind="ExternalInput")
    xT8 = nc.dram_tensor("xT8", (D, T), FP8, kind="ExternalInput")
    wgb = nc.dram_tensor("wgb", (2, D, HU), BF16, kind="ExternalInput")
    wub = nc.dram_tensor("wub", (2, D, HU), BF16, kind="ExternalInput")
    wdb = nc.dram_tensor("wdb", (2, HU, D), BF16, kind="ExternalInput")
    wg8 = nc.dram_tensor("wg8", (E, D, HU), FP8, kind="ExternalInput")
    wu8 = nc.dram_tensor("wu8", (E, D, HU), FP8, kind="ExternalInput")
    wd8 = nc.dram_tensor("wd8", (E, HU, D), FP8, kind="ExternalInput")
    gb = nc.dram_tensor("gb", (128, NU, NH), F32, kind="ExternalInput")
    ub = nc.dram_tensor("ub", (128, NU, NH), F32, kind="ExternalInput")
    rw = nc.dram_tensor("rw", (128, NU, NTT), F32, kind="ExternalInput")
    cv = nc.dram_tensor("cv", (128, ND, NTT), F32, kind="ExternalInput")
    outT = nc.dram_tensor("outT", (D, T), F32, kind="ExternalOutput")

    with tile.TileContext(nc) as tc:
        with (
            tc.tile_pool(name="persist", bufs=1) as persist,
            tc.tile_pool(name="wpool", bufs=2) as wpool,
            tc.tile_pool(name="hpool", bufs=2) as hpool,
            tc.tile_pool(name="spool", bufs=4) as spool,
            tc.tile_pool(name="dpool", bufs=4) as dpool,
            tc.tile_pool(name="gups", bufs=2, space="PSUM") as gups,
            tc.tile_pool(name="ops", bufs=1, space="PSUM") as opsp,
        ):
            xb = persist.tile([128, NK, T], BF16)
            x8t = persist.tile([128, NK, T], FP8)
            gbt = persist.tile([128, NU, NH], F32)
            ubt = persist.tile([128, NU, NH], F32)
            rwt = persist.tile([128, NU, NTT], F32)
            cvt = persist.tile([128, ND, NTT], F32)

            def load_unit_weights(t, u, wgt, wut, wdt, first=False):
                fp8u = UNIT_FP8[u]
                wgd, wud, wdd = (wg8, wu8, wd8) if fp8u else (wgb, wub, wdb)
                ui = u - 2 if fp8u else u
                # split gate/up loads by h-half so the first matmuls of a unit
                # only wait on the slices they read; interleave the very first
                # unit's loads with the x/table loads it needs.
                for half in range(2):
                    hh = slice(half * 512, (half + 1) * 512)
                    for k in range(NK):
                        nc.gpsimd.dma_start(wgt[:, k, hh],
                                            wgd.ap()[ui, k * 128:(k + 1) * 128, hh])
                        if first:
                            nc.sync.dma_start(
                                xb[:, k, slice(half * TT, (half + 1) * TT)],
                                xTb.ap()[k * 128:(k + 1) * 128,
                                         half * TT:(half + 1) * TT])
                            nc.sync.dma_start(
                                x8t[:, k, slice(half * TT, (half + 1) * TT)],
                                xT8.ap()[k * 128:(k + 1) * 128,
                                         half * TT:(half + 1) * TT])
                    if first and half == 0:
                        nc.sync.dma_start(gbt[:], gb.ap()[:])
                        nc.sync.dma_start(ubt[:], ub.ap()[:])
                        nc.sync.dma_start(rwt[:], rw.ap()[:])
                        nc.sync.dma_start(cvt[:], cv.ap()[:])
                    for k in range(NK):
                        nc.gpsimd.dma_start(wut[:, k, hh],
                                            wud.ap()[ui, k * 128:(k + 1) * 128, hh])
                for k in range(NH):
                    nc.gpsimd.dma_start(wdt[:, k, :],
                                        wdd.ap()[ui, k * 128:(k + 1) * 128, :])
                if first:
                    # remaining x token tiles (the first unit only reads tile 0)
                    for tt in range(2, NTT):
                        tks = slice(tt * TT, (tt + 1) * TT)
                        for k in range(NK):
                            nc.sync.dma_start(
                                xb[:, k, tks],
                                xTb.ap()[k * 128:(k + 1) * 128, tks])
                            nc.sync.dma_start(
                                x8t[:, k, tks],
                                xT8.ap()[k * 128:(k + 1) * 128, tks])

            def unit_tiles(u):
                fp8u = UNIT_FP8[u]
                dt_ = FP8 if fp8u else BF16
                sfx = "8" if fp8u else "b"
                wgt = wpool.tile([128, NK, HU], dt_, tag=f"wg{sfx}")
                wut = wpool.tile([128, NK, HU], dt_, tag=f"wu{sfx}")
                wdt = wpool.tile([128, NH, D], dt_, tag=f"wd{sfx}")
                return wgt, wut, wdt

            first = True
            for t in range(NTT):
                tok = slice(t * TT, (t + 1) * TT)
                odst = [opsp.tile([128, TT], F32, tag=f"o{di}",
                                  name=f"o{di}_t{t}") for di in range(ND)]
                for u in range(NU):
                    fp8u = UNIT_FP8[u]
                    wgt, wut, wdt = unit_tiles(u)
                    load_unit_weights(t, u, wgt, wut, wdt, first=first)
                    first = False

                    hts = hpool.tile([128, NH, TT], FP8 if fp8u else BF16,
                                     tag="h8" if fp8u else "hb",
                                     name=f"h_u{u}t{t}")
                    for hi in range(NH):
                        hc = slice(hi * 128, (hi + 1) * 128)
                        gps = gups.tile([128, TT], F32, tag="g")
                        if fp8u:
                            for kp in range(NK // 2):
                                nc.tensor.matmul(
                                    gps[:], wgt[:, 2 * kp:2 * kp + 2, hc],
                                    x8t[:, 2 * kp:2 * kp + 2, tok],
                                    start=(kp == 0), stop=(kp == NK // 2 - 1),
                                    perf_mode=DR)
                        else:
                            for k in range(NK):
                                nc.tensor.matmul(
                                    gps[:], wgt[:, k, hc], xb[:, k, tok],
                                    start=(k == 0), stop=(k == NK - 1))
                        ups = gups.tile([128, TT], F32, tag="u")
                        if fp8u:
                            for kp in range(NK // 2):
                                nc.tensor.matmul(
                                    ups[:], wut[:, 2 * kp:2 * kp + 2, hc],
                                    x8t[:, 2 * kp:2 * kp + 2, tok],
                                    start=(kp == 0), stop=(kp == NK // 2 - 1),
                                    perf_mode=DR)
                        else:
                            for k in range(NK):
                                nc.tensor.matmul(
                                    ups[:], wut[:, k, hc], xb[:, k, tok],
                                    start=(k == 0), stop=(k == NK - 1))
                        sg = spool.tile([128, TT], F32, tag="sg")
                        nc.scalar.activation(sg[:], gps[:], ACT,
                                             bias=gbt[:, u, hi:hi + 1],
                                             scale=(1.0 / SXW) if fp8u else 1.0)
                        su = spool.tile([128, TT], F32, tag="su")
                        nc.vector.tensor_scalar(su[:], ups[:],
                                                ubt[:, u, hi:hi + 1],
                                                rwt[:, u, t:t + 1],
                                                Alu.add, Alu.mult)
                        nc.vector.tensor_tensor(hts[:, hi, :], sg[:], su[:],
                                                Alu.mult)

                    if fp8u:
                        for kp in range(NH // 2):
                            for di in range(ND):
                                nc.tensor.matmul(
                                    odst[di][:],
                                    wdt[:, 2 * kp:2 * kp + 2,
                                        di * 128:(di + 1) * 128],
                                    hts[:, 2 * kp:2 * kp + 2, :],
                                    start=(u == 0 and kp == 0),
                                    stop=(u == NU - 1 and kp == NH // 2 - 1),
                                    perf_mode=DR, skip_group_check=True)
                    else:
                        for k in range(NH):
                            for di in range(ND):
                                nc.tensor.matmul(
                                    odst[di][:],
                                    wdt[:, k, di * 128:(di + 1) * 128],
                                    hts[:, k, :],
                                    start=(u == 0 and k == 0),
                                    stop=(u == NU - 1 and k == NH - 1),
                                    skip_group_check=True)

                for di in range(ND):
                    dtl = dpool.tile([128, TT], F32, tag="d")
                    nc.vector.tensor_scalar(dtl[:], odst[di][:],
                                            cvt[:, di, t:t + 1], 1.0 / SDC,
                                            Alu.add, Alu.mult)
                    nc.sync.dma_start(outT.ap()[di * 128:(di + 1) * 128, tok],
                                      dtl[:])
    nc.compile()
    return nc


_NC = None


def _get_nc():
    global _NC
    if _NC is None:
        _NC = _build()
    return _NC


def _bf16(a):
    return np.ascontiguousarray(np.asarray(a, np.float32)).astype(ml_dtypes.bfloat16)


def _fp8(a, scale):
    return np.ascontiguousarray(
        np.asarray(a, np.float32) * scale).astype(ml_dtypes.float8_e4m3)


def _colmaj(v):
    return np.asarray(v, np.float32).reshape(-1, 128).T


def _pack_shared(Ws_gate, bs_gate, Ws_up, bs_up, Ws_down, bs_down,
                 Wr_gate, br_gate, Wr_up, br_up, Wr_down, br_down):
    wgb = np.empty((2, D, HU), ml_dtypes.bfloat16)
    wub = np.empty((2, D, HU), ml_dtypes.bfloat16)
    wdb = np.empty((2, HU, D), ml_dtypes.bfloat16)
    gbt = np.empty((128, NU, NH), np.float32)
    ubt = np.empty((128, NU, NH), np.float32)
    for u in range(2):
        h0 = slice(u * HU, (u + 1) * HU)
        wgb[u] = _bf16(Ws_gate[:, h0])
        wub[u] = _bf16(Ws_up[:, h0])
        wdb[u] = _bf16(Ws_down[h0, :])
        gbt[:, u, :] = _colmaj(bs_gate[h0])
        ubt[:, u, :] = _colmaj(bs_up[h0])
    wg8 = _fp8(Wr_gate, SWQ)
    wu8 = _fp8(Wr_up, SWQ)
    wd8 = _fp8(Wr_down, SWQ)
    for e in range(E):
        gbt[:, 2 + e, :] = _colmaj(br_gate[e])
        ubt[:, 2 + e, :] = _colmaj(br_up[e]) * SXW
    return wgb, wub, wdb, wg8, wu8, wd8, gbt, ubt


def _run(inputs, trace=False):
    x = np.asarray(inputs["x"], np.float32)
    rweights = np.asarray(inputs["routing_weights"], np.float32)
    wgb, wub, wdb, wg8, wu8, wd8, gbt, ubt = _pack_shared(
        np.asarray(inputs["Ws_gate"], np.float32), inputs["bs_gate"],
        np.asarray(inputs["Ws_up"], np.float32), inputs["bs_up"],
        np.asarray(inputs["Ws_down"], np.float32), inputs["bs_down"],
        np.asarray(inputs["Wr_gate"], np.float32), inputs["br_gate"],
        np.asarray(inputs["Wr_up"], np.float32), inputs["br_up"],
        np.asarray(inputs["Wr_down"], np.float32), inputs["br_down"])
    bs_down = np.asarray(inputs["bs_down"], np.float32)
    br_down = np.asarray(inputs["br_down"], np.float32)
    # down-bias vector per batch: bs_down + sum_e rw[b,e]*br_down[e]
    cfull = bs_down[None, :] + rweights @ br_down       # [B, D]

    in_maps = []
    for i in range(NCORES):
        xT = x[BL * i:BL * (i + 1)].reshape(T, D).T
        rwtab = np.empty((128, NU, NTT), np.float32)
        rwtab[:, :2, :] = SDC
        cvtab = np.empty((128, ND, NTT), np.float32)
        for t in range(NTT):
            bg = BL * i + t // (K // TT)
            for e in range(E):
                rwtab[:, 2 + e, t] = rweights[bg, e] * (C / SXW)
            cvtab[:, :, t] = cfull[bg].reshape(ND, 128).T * SDC
        in_maps.append({"xTb": _bf16(xT), "xT8": _fp8(xT, SX),
                        "wgb": wgb, "wub": wub, "wdb": wdb,
                        "wg8": wg8, "wu8": wu8, "wd8": wd8,
                        "gb": gbt, "ub": ubt,
                        "rw": np.ascontiguousarray(rwtab),
                        "cv": np.ascontiguousarray(cvtab)})

    res = bass_utils.run_bass_kernel_spmd(_get_nc(), in_maps,
                                          core_ids=list(range(NCORES)),
                                          trace=trace)
    out = np.empty((B, K, D), np.float32)
    for i in range(NCORES):
        out[BL * i:BL * (i + 1)] = res.results[i]["outT"].T.reshape(BL, K, D)
    return out, res


def kernel(**inputs) -> np.ndarray:
    out, _ = _run(inputs, trace=False)
    return out


# revision 12
# speedup vs baseline: 1.4170x; 1.1069x over previous
"""MoE FFN (shared SwiGLU + 8 dense-routed SwiGLU experts) on 8 TRN2 NeuronCores.

Strategy: data-parallel over batch (B=16 -> 2 batches/core). The 10 uniform
512->1024->512 SwiGLU units (2 shared halves + 8 experts) run with per-unit
precision: shared units in bf16, expert units in fp8e4m3 DoubleRow matmuls
(2x PE throughput). Expert quantization errors are independent across the 8
experts and average down under the routing weights, keeping total rel err
~1.8e-2 (<2e-2 gate) while cutting PE time to ~0.6x of the bf16 roofline.

All weights stay resident in SBUF (~170KB/partition incl. x), host-repacked
to partition-contiguous layout so the whole working set loads with ~50 large
DMA descriptors (the v2 kernel spent 712us of gpsimd time issuing 960
fine-grained weight DMAs). Loop is token-tile-outer / unit-inner so all 10
units' down-projections accumulate directly in PSUM at a common scale
(shared h pre-scaled by SW*C=32768, exact in floating point); one vector op
per (d-tile, token-tile) drains PSUM with bias + descale fused. The up-branch
tensor_scalar runs on the gpsimd (pool) engine to keep DVE off the critical
path.
"""
import sys

if "/opt/trn_rl_repo" not in sys.path:
    sys.path.insert(0, "/opt/trn_rl_repo")

import numpy as np
import ml_dtypes

import concourse.bass as bass  # noqa: F401  (registers engine classes)
import concourse.tile as tile
from concourse import bacc, mybir
from concourse import bass_utils

F32 = mybir.dt.float32
BF16 = mybir.dt.bfloat16
FP8 = mybir.dt.float8e4
Silu = mybir.ActivationFunctionType.Silu
ACT = Silu  # debug harnesses may swap to Sigmoid (CoreSim lacks Silu)
Alu = mybir.AluOpType
DR = mybir.MatmulPerfMode.DoubleRow

B, K, D = 16, 1024, 512
HS, HR, E = 2048, 1024, 8
NCORES = 8
BL = B // NCORES          # batches per core = 2
T = BL * K                # tokens per core = 2048
TT = 512                  # token tile (matmul moving dim)
NTT = T // TT             # 4 token tiles per core
NU = 2 + E                # units: 2 shared halves + 8 experts
HU = 1024                 # hidden width of every unit
NH = HU // 128            # 8 h-tiles per unit
ND = D // 128             # 4 d-tiles
NK = D // 128             # 4 contraction tiles for gate/up
WSZ = NK * HU             # per-matrix elements per partition (4096)

SX = 32.0                 # fp8 x scale
SWQ = 1024.0              # fp8 weight scale
C = 32.0                  # h-domain scale for expert fp8 h
SXW = SX * SWQ            # expert gate/up psum scale
SDC = SWQ * C             # common down psum scale (all units)

# per-unit precision: shared halves bf16, experts fp8
UNIT_FP8 = [False, False] + [True] * E


def _build():
    nc = bacc.Bacc("TRN2", target_bir_lowering=False, debug=False,
                   num_devices=NCORES)
    # weights packed host-side to SBUF layout: partition-contiguous, one
    # (unit, matrix) block of WSZ elements per partition per block.
    wsb = nc.dram_tensor("wsb", (128, 2 * 3 * WSZ), BF16, kind="ExternalInput")
    we8 = nc.dram_tensor("we8", (128, E * 3 * WSZ), FP8, kind="ExternalInput")
    xTb = nc.dram_tensor("xTb", (128, NK * T), BF16, kind="ExternalInput")
    xT8 = nc.dram_tensor("xT8", (128, NK * T), FP8, kind="ExternalInput")
    gb = nc.dram_tensor("gb", (128, NU, NH), F32, kind="ExternalInput")
    ub = nc.dram_tensor("ub", (128, NU, NH), F32, kind="ExternalInput")
    rw = nc.dram_tensor("rw", (128, NU, NTT), F32, kind="ExternalInput")
    cv = nc.dram_tensor("cv", (128, ND, NTT), F32, kind="ExternalInput")
    outT = nc.dram_tensor("outT", (D, T), F32, kind="ExternalOutput")

    with tile.TileContext(nc) as tc:
        with (
            tc.tile_pool(name="persist", bufs=1) as persist,
            tc.tile_pool(name="hpool", bufs=1) as hpool,
            tc.tile_pool(name="spool", bufs=2) as spool,
            tc.tile_pool(name="dpool", bufs=2) as dpool,
            tc.tile_pool(name="gups", bufs=2, space="PSUM") as gups,
            tc.tile_pool(name="ops", bufs=1, space="PSUM") as opsp,
        ):
            xb = persist.tile([128, NK, T], BF16)
            x8t = persist.tile([128, NK, T], FP8)
            gbt = persist.tile([128, NU, NH], F32)
            ubt = persist.tile([128, NU, NH], F32)
            rwt = persist.tile([128, NU, NTT], F32)
            cvt = persist.tile([128, ND, NTT], F32)

            # resident weights: one tile triple per unit, one DMA per matrix
            wtiles = []
            for u in range(NU):
                fp8u = UNIT_FP8[u]
                dt_ = FP8 if fp8u else BF16
                src = we8 if fp8u else wsb
                base = (u - 2 if fp8u else u) * 3 * WSZ
                wgt = persist.tile([128, NK, HU], dt_, name=f"wg{u}")
                wut = persist.tile([128, NK, HU], dt_, name=f"wu{u}")
                wdt = persist.tile([128, NH, D], dt_, name=f"wd{u}")
                wtiles.append((wgt, wut, wdt))
                if u == 0:
                    # x + tables on the sync queue, in parallel with the
                    # first unit's weights on the gpsimd queue
                    nc.sync.dma_start(xb[:], xTb.ap()[:])
                    nc.sync.dma_start(x8t[:], xT8.ap()[:])
                    nc.sync.dma_start(gbt[:], gb.ap()[:])
                    nc.sync.dma_start(ubt[:], ub.ap()[:])
                    nc.sync.dma_start(rwt[:], rw.ap()[:])
                    nc.sync.dma_start(cvt[:], cv.ap()[:])
                nc.gpsimd.dma_start(wgt[:], src.ap()[:, base:base + WSZ])
                nc.gpsimd.dma_start(wut[:], src.ap()[:, base + WSZ:base + 2 * WSZ])
                nc.gpsimd.dma_start(wdt[:], src.ap()[:, base + 2 * WSZ:base + 3 * WSZ])

            for t in range(NTT):
                tok = slice(t * TT, (t + 1) * TT)
                odst = [opsp.tile([128, TT], F32, tag=f"o{di}",
                                  name=f"o{di}_t{t}") for di in range(ND)]
                for u in range(NU):
                    fp8u = UNIT_FP8[u]
                    wgt, wut, wdt = wtiles[u]
                    sdt = BF16 if fp8u else F32
                    hts = hpool.tile([128, NH, TT], FP8 if fp8u else BF16,
                                     tag="h8" if fp8u else "hb",
                                     name=f"h_u{u}t{t}")
                    for hi in range(NH):
                        hc = slice(hi * 128, (hi + 1) * 128)
                        gps = gups.tile([128, TT], F32, tag="g")
                        if fp8u:
                            for kp in range(NK // 2):
                                nc.tensor.matmul(
                                    gps[:], wgt[:, 2 * kp:2 * kp + 2, hc],
                                    x8t[:, 2 * kp:2 * kp + 2, tok],
                                    start=(kp == 0), stop=(kp == NK // 2 - 1),
                                    perf_mode=DR)
                        else:
                            for k in range(NK):
                                nc.tensor.matmul(
                                    gps[:], wgt[:, k, hc], xb[:, k, tok],
                                    start=(k == 0), stop=(k == NK - 1))
                        ups = gups.tile([128, TT], F32, tag="u")
                        if fp8u:
                            for kp in range(NK // 2):
                                nc.tensor.matmul(
                                    ups[:], wut[:, 2 * kp:2 * kp + 2, hc],
                                    x8t[:, 2 * kp:2 * kp + 2, tok],
                                    start=(kp == 0), stop=(kp == NK // 2 - 1),
                                    perf_mode=DR)
                        else:
                            for k in range(NK):
                                nc.tensor.matmul(
                                    ups[:], wut[:, k, hc], xb[:, k, tok],
                                    start=(k == 0), stop=(k == NK - 1))
                        sg = spool.tile([128, TT], sdt, tag="sg8" if fp8u else "sgb")
                        nc.scalar.activation(sg[:], gps[:], ACT,
                                             bias=gbt[:, u, hi:hi + 1],
                                             scale=(1.0 / SXW) if fp8u else 1.0)
                        su = spool.tile([128, TT], sdt, tag="su8" if fp8u else "sub")
                        nc.vector.tensor_scalar(su[:], ups[:],
                                                ubt[:, u, hi:hi + 1],
                                                rwt[:, u, t:t + 1],
                                                Alu.add, Alu.mult)
                        nc.gpsimd.tensor_tensor(hts[:, hi, :], sg[:], su[:],
                                                Alu.mult)

                    if fp8u:
                        for kp in range(NH // 2):
                            for di in range(ND):
                                nc.tensor.matmul(
                                    odst[di][:],
                                    wdt[:, 2 * kp:2 * kp + 2,
                                        di * 128:(di + 1) * 128],
                                    hts[:, 2 * kp:2 * kp + 2, :],
                                    start=(u == 0 and kp == 0),
                                    stop=(u == NU - 1 and kp == NH // 2 - 1),
                                    perf_mode=DR, skip_group_check=True)
                    else:
                        for k in range(NH):
                            for di in range(ND):
                                nc.tensor.matmul(
                                    odst[di][:],
                                    wdt[:, k, di * 128:(di + 1) * 128],
                                    hts[:, k, :],
                                    start=(u == 0 and k == 0),
                                    stop=(u == NU - 1 and k == NH - 1),
                                    skip_group_check=True)

                for di in range(ND):
                    dtl = dpool.tile([128, TT], F32, tag="d")
                    nc.scalar.activation(dtl[:], odst[di][:],
                                         mybir.ActivationFunctionType.Identity,
                                         bias=cvt[:, di, t:t + 1],
                                         scale=1.0 / SDC)
                    nc.sync.dma_start(outT.ap()[di * 128:(di + 1) * 128, tok],
                                      dtl[:])
    nc.compile()
    return nc


_NC = None


def _get_nc():
    global _NC
    if _NC is None:
        _NC = _build()
    return _NC


def _bf16(a):
    return np.ascontiguousarray(np.asarray(a, np.float32)).astype(ml_dtypes.bfloat16)


def _fp8(a, scale):
    return np.ascontiguousarray(
        np.asarray(a, np.float32) * scale).astype(ml_dtypes.float8_e4m3)


def _colmaj(v):
    return np.asarray(v, np.float32).reshape(-1, 128).T


def _sbufpack(w):
    """[D_in, D_out] -> [128, (D_in/128)*D_out] partition-contiguous."""
    din, dout = w.shape
    return w.reshape(din // 128, 128, dout).transpose(1, 0, 2).reshape(128, -1)


def _pack_shared(Ws_gate, bs_gate, Ws_up, bs_up, Ws_down, bs_down,
                 Wr_gate, br_gate, Wr_up, br_up, Wr_down, br_down):
    ws = np.empty((128, 2 * 3 * WSZ), np.float32)
    we = np.empty((128, E * 3 * WSZ), np.float32)
    gbt = np.empty((128, NU, NH), np.float32)
    ubt = np.empty((128, NU, NH), np.float32)
    for u in range(2):
        h0 = slice(u * HU, (u + 1) * HU)
        base = u * 3 * WSZ
        ws[:, base:base + WSZ] = _sbufpack(np.asarray(Ws_gate, np.float32)[:, h0])
        ws[:, base + WSZ:base + 2 * WSZ] = _sbufpack(
            np.asarray(Ws_up, np.float32)[:, h0])
        ws[:, base + 2 * WSZ:base + 3 * WSZ] = _sbufpack(
            np.asarray(Ws_down, np.float32)[h0, :])
        gbt[:, u, :] = _colmaj(bs_gate[h0])
        ubt[:, u, :] = _colmaj(bs_up[h0])
    for e in range(E):
        base = e * 3 * WSZ
        we[:, base:base + WSZ] = _sbufpack(np.asarray(Wr_gate, np.float32)[e])
        we[:, base + WSZ:base + 2 * WSZ] = _sbufpack(
            np.asarray(Wr_up, np.float32)[e])
        we[:, base + 2 * WSZ:base + 3 * WSZ] = _sbufpack(
            np.asarray(Wr_down, np.float32)[e])
        gbt[:, 2 + e, :] = _colmaj(br_gate[e])
        ubt[:, 2 + e, :] = _colmaj(br_up[e]) * SXW
    return _bf16(ws), _fp8(we, SWQ), gbt, ubt


def _run(inputs, trace=False):
    x = np.asarray(inputs["x"], np.float32)
    rweights = np.asarray(inputs["routing_weights"], np.float32)
    wsb, we8, gbt, ubt = _pack_shared(
        np.asarray(inputs["Ws_gate"], np.float32), inputs["bs_gate"],
        np.asarray(inputs["Ws_up"], np.float32), inputs["bs_up"],
        np.asarray(inputs["Ws_down"], np.float32), inputs["bs_down"],
        np.asarray(inputs["Wr_gate"], np.float32), inputs["br_gate"],
        np.asarray(inputs["Wr_up"], np.float32), inputs["br_up"],
        np.asarray(inputs["Wr_down"], np.float32), inputs["br_down"])
    bs_down = np.asarray(inputs["bs_down"], np.float32)
    br_down = np.asarray(inputs["br_down"], np.float32)
    # down-bias vector per batch: bs_down + sum_e rw[b,e]*br_down[e]
    cfull = bs_down[None, :] + rweights @ br_down       # [B, D]

    in_maps = []
    for i in range(NCORES):
        xT = x[BL * i:BL * (i + 1)].reshape(T, D).T     # [D, T]
        # pack x to [128, NK*T]: partition p, block k, token t = xT[k*128+p, t]
        xP = xT.reshape(NK, 128, T).transpose(1, 0, 2).reshape(128, NK * T)
        rwtab = np.empty((128, NU, NTT), np.float32)
        rwtab[:, :2, :] = SDC
        cvtab = np.empty((128, ND, NTT), np.float32)
        for t in range(NTT):
            bg = BL * i + t // (K // TT)
            for e in range(E):
                rwtab[:, 2 + e, t] = rweights[bg, e] * (C / SXW)
            cvtab[:, :, t] = cfull[bg].reshape(ND, 128).T
        in_maps.append({"xTb": _bf16(xP), "xT8": _fp8(xP, SX),
                        "wsb": wsb, "we8": we8,
                        "gb": gbt, "ub": ubt,
                        "rw": np.ascontiguousarray(rwtab),
                        "cv": np.ascontiguousarray(cvtab)})

    res = bass_utils.run_bass_kernel_spmd(_get_nc(), in_maps,
                                          core_ids=list(range(NCORES)),
                                          trace=trace)
    out = np.empty((B, K, D), np.float32)
    for i in range(NCORES):
        out[BL * i:BL * (i + 1)] = res.results[i]["outT"].T.reshape(BL, K, D)
    return out, res


def kernel(**inputs) -> np.ndarray:
    out, _ = _run(inputs, trace=False)
    return out


# revision 19
# speedup vs baseline: 1.5885x; 1.1211x over previous
"""MoE FFN (shared SwiGLU + 8 dense-routed SwiGLU experts) on 8 TRN2 NeuronCores.

Strategy: data-parallel over batch (B=16 -> 2 batches/core). The 10 uniform
512->1024->512 SwiGLU units (2 shared halves + 8 experts) run with per-unit
precision: shared units in bf16, expert units in fp8e4m3 DoubleRow matmuls
(2x PE throughput). Expert quantization errors are independent across the 8
experts and average down under the routing weights, keeping total rel err
~1.8e-2 (<2e-2 gate) while cutting PE time to ~0.6x of the bf16 roofline.

All weights stay resident in SBUF (~170KB/partition incl. x), host-repacked
to partition-contiguous layout so the whole working set loads with ~50 large
DMA descriptors (the v2 kernel spent 712us of gpsimd time issuing 960
fine-grained weight DMAs). Loop is token-tile-outer / unit-inner so all 10
units' down-projections accumulate directly in PSUM at a common scale
(shared h pre-scaled by SW*C=32768, exact in floating point); one vector op
per (d-tile, token-tile) drains PSUM with bias + descale fused. The up-branch
tensor_scalar runs on the gpsimd (pool) engine to keep DVE off the critical
path.
"""
import sys

if "/opt/trn_rl_repo" not in sys.path:
    sys.path.insert(0, "/opt/trn_rl_repo")

import numpy as np
import ml_dtypes

import concourse.bass as bass  # noqa: F401  (registers engine classes)
import concourse.tile as tile
from concourse import bacc, mybir
from concourse import bass_utils

F32 = mybir.dt.float32
BF16 = mybir.dt.bfloat16
FP8 = mybir.dt.float8e4
Silu = mybir.ActivationFunctionType.Silu
ACT = Silu  # debug harnesses may swap to Sigmoid (CoreSim lacks Silu)
Alu = mybir.AluOpType
DR = mybir.MatmulPerfMode.DoubleRow

B, K, D = 16, 1024, 512
HS, HR, E = 2048, 1024, 8
NCORES = 8
BL = B // NCORES          # batches per core = 2
T = BL * K                # tokens per core = 2048
TT = 512                  # token tile (matmul moving dim)
NTT = T // TT             # 4 token tiles per core
NU = 2 + E                # units: 2 shared halves + 8 experts
HU = 1024                 # hidden width of every unit
NH = HU // 128            # 8 h-tiles per unit
ND = D // 128             # 4 d-tiles
NK = D // 128             # 4 contraction tiles for gate/up
WSZ = NK * HU             # per-matrix elements per partition (4096)

SX = 32.0                 # fp8 x scale
SWQ = 1024.0              # fp8 weight scale
C = 32.0                  # h-domain scale for expert fp8 h
SXW = SX * SWQ            # expert gate/up psum scale
SDC = SWQ * C             # common down psum scale (all units)

# per-unit precision: shared halves bf16, experts fp8
UNIT_FP8 = [False, False] + [True] * E

# When every up-projection bias is zero (true for this problem's inputs),
# the up-branch scale and the h product fuse into one DVE op:
#   h = (ups * rwC) * silu(g).
# The general path (ts on vector + tt on gpsimd) stays available for
# nonzero biases; _run picks at call time.
_FUSED = True


def _build(fused):
    nc = bacc.Bacc("TRN2", target_bir_lowering=False, debug=False,
                   num_devices=NCORES)
    # weights packed host-side to SBUF layout: partition-contiguous, one
    # (unit, matrix) block of WSZ elements per partition per block.
    wsb = nc.dram_tensor("wsb", (128, 2 * 3 * WSZ), BF16, kind="ExternalInput")
    we8 = nc.dram_tensor("we8", (128, E * 3 * WSZ), FP8, kind="ExternalInput")
    xTb = nc.dram_tensor("xTb", (128, NK, T), BF16, kind="ExternalInput")
    xT8 = nc.dram_tensor("xT8", (128, NK, T), FP8, kind="ExternalInput")
    gb = nc.dram_tensor("gb", (128, NU, NH), F32, kind="ExternalInput")
    ub = nc.dram_tensor("ub", (128, NU, NH), F32, kind="ExternalInput")
    rw = nc.dram_tensor("rw", (128, NU, NTT), F32, kind="ExternalInput")
    cv = nc.dram_tensor("cv", (128, ND, NTT), F32, kind="ExternalInput")
    outT = nc.dram_tensor("outT", (D, T), F32, kind="ExternalOutput")

    with tile.TileContext(nc) as tc:
        with (
            tc.tile_pool(name="persist", bufs=1) as persist,
            tc.tile_pool(name="hpool", bufs=1) as hpool,
            tc.tile_pool(name="spool", bufs=2) as spool,
            tc.tile_pool(name="dpool", bufs=2) as dpool,
            tc.tile_pool(name="gups", bufs=2, space="PSUM") as gups,
            tc.tile_pool(name="ops", bufs=1, space="PSUM") as opsp,
        ):
            xb = persist.tile([128, NK, T], BF16)
            x8t = persist.tile([128, NK, T], FP8)
            gbt = persist.tile([128, NU, NH], F32)
            ubt = persist.tile([128, NU, NH], F32)
            rwt = persist.tile([128, NU, NTT], F32)
            cvt = persist.tile([128, ND, NTT], F32)

            # resident weights: one tile triple per unit, one DMA per matrix
            wtiles = []
            for u in range(NU):
                fp8u = UNIT_FP8[u]
                dt_ = FP8 if fp8u else BF16
                src = we8 if fp8u else wsb
                base = (u - 2 if fp8u else u) * 3 * WSZ
                wgt = persist.tile([128, NK, HU], dt_, name=f"wg{u}")
                wut = persist.tile([128, NK, HU], dt_, name=f"wu{u}")
                wdt = persist.tile([128, NH, D], dt_, name=f"wd{u}")
                wtiles.append((wgt, wut, wdt))
                if u == 0:
                    # x + tables on the sync queue, in parallel with the
                    # first unit's weights on the gpsimd queue; the first
                    # token tile lands first so unit 0 can start early
                    nc.sync.dma_start(xb[:, :, 0:TT], xTb.ap()[:, :, 0:TT])
                    nc.sync.dma_start(x8t[:, :, 0:TT], xT8.ap()[:, :, 0:TT])
                    nc.sync.dma_start(gbt[:], gb.ap()[:])
                    nc.sync.dma_start(ubt[:], ub.ap()[:])
                    nc.sync.dma_start(rwt[:], rw.ap()[:])
                    nc.sync.dma_start(cvt[:], cv.ap()[:])
                    nc.sync.dma_start(xb[:, :, TT:], xTb.ap()[:, :, TT:])
                    nc.sync.dma_start(x8t[:, :, TT:], xT8.ap()[:, :, TT:])
                nc.gpsimd.dma_start(wgt[:], src.ap()[:, base:base + WSZ])
                nc.gpsimd.dma_start(wut[:], src.ap()[:, base + WSZ:base + 2 * WSZ])
                nc.gpsimd.dma_start(wdt[:], src.ap()[:, base + 2 * WSZ:base + 3 * WSZ])

            for t in range(NTT):
                tok = slice(t * TT, (t + 1) * TT)
                odst = [opsp.tile([128, TT], F32, tag=f"o{di}",
                                  name=f"o{di}_t{t}") for di in range(ND)]
                for u in range(NU):
                    fp8u = UNIT_FP8[u]
                    wgt, wut, wdt = wtiles[u]
                    sdt = BF16 if fp8u else F32
                    hts = hpool.tile([128, NH, TT], FP8 if fp8u else BF16,
                                     tag="h8" if fp8u else "hb",
                                     name=f"h_u{u}t{t}")
                    for hi in range(NH):
                        hc = slice(hi * 128, (hi + 1) * 128)
                        gps = gups.tile([128, TT], F32, tag="g")
                        if fp8u:
                            for kp in range(NK // 2):
                                nc.tensor.matmul(
                                    gps[:], wgt[:, 2 * kp:2 * kp + 2, hc],
                                    x8t[:, 2 * kp:2 * kp + 2, tok],
                                    start=(kp == 0), stop=(kp == NK // 2 - 1),
                                    perf_mode=DR)
                        else:
                            for k in range(NK):
                                nc.tensor.matmul(
                                    gps[:], wgt[:, k, hc], xb[:, k, tok],
                                    start=(k == 0), stop=(k == NK - 1))
                        ups = gups.tile([128, TT], F32, tag="u")
                        if fp8u:
                            for kp in range(NK // 2):
                                nc.tensor.matmul(
                                    ups[:], wut[:, 2 * kp:2 * kp + 2, hc],
                                    x8t[:, 2 * kp:2 * kp + 2, tok],
                                    start=(kp == 0), stop=(kp == NK // 2 - 1),
                                    perf_mode=DR)
                        else:
                            for k in range(NK):
                                nc.tensor.matmul(
                                    ups[:], wut[:, k, hc], xb[:, k, tok],
                                    start=(k == 0), stop=(k == NK - 1))
                        sg = spool.tile([128, TT], sdt, tag="sg8" if fp8u else "sgb")
                        nc.scalar.activation(sg[:], gps[:], ACT,
                                             bias=gbt[:, u, hi:hi + 1],
                                             scale=(1.0 / SXW) if fp8u else 1.0)
                        if fused:
                            nc.vector.scalar_tensor_tensor(
                                hts[:, hi, :], ups[:], rwt[:, u, t:t + 1],
                                sg[:], Alu.mult, Alu.mult)
                        else:
                            su = spool.tile([128, TT], sdt,
                                            tag="su8" if fp8u else "sub")
                            nc.vector.tensor_scalar(su[:], ups[:],
                                                    ubt[:, u, hi:hi + 1],
                                                    rwt[:, u, t:t + 1],
                                                    Alu.add, Alu.mult)
                            nc.gpsimd.tensor_tensor(hts[:, hi, :], sg[:],
                                                    su[:], Alu.mult)

                    if fp8u:
                        for kp in range(NH // 2):
                            for di in range(ND):
                                nc.tensor.matmul(
                                    odst[di][:],
                                    wdt[:, 2 * kp:2 * kp + 2,
                                        di * 128:(di + 1) * 128],
                                    hts[:, 2 * kp:2 * kp + 2, :],
                                    start=(u == 0 and kp == 0),
                                    stop=(u == NU - 1 and kp == NH // 2 - 1),
                                    perf_mode=DR, skip_group_check=True)
                    else:
                        for k in range(NH):
                            for di in range(ND):
                                nc.tensor.matmul(
                                    odst[di][:],
                                    wdt[:, k, di * 128:(di + 1) * 128],
                                    hts[:, k, :],
                                    start=(u == 0 and k == 0),
                                    stop=(u == NU - 1 and k == NH - 1),
                                    skip_group_check=True)

                for di in range(ND):
                    dtl = dpool.tile([128, TT], F32, tag="d")
                    nc.scalar.activation(dtl[:], odst[di][:],
                                         mybir.ActivationFunctionType.Identity,
                                         bias=cvt[:, di, t:t + 1],
                                         scale=1.0 / SDC)
                    nc.sync.dma_start(outT.ap()[di * 128:(di + 1) * 128, tok],
                                      dtl[:])
    nc.compile()
    return nc


_NC = {}


def _get_nc(fused):
    if fused not in _NC:
        _NC[fused] = _build(fused)
    return _NC[fused]


def _bf16(a):
    return np.ascontiguousarray(np.asarray(a, np.float32)).astype(ml_dtypes.bfloat16)


def _fp8(a, scale):
    return np.ascontiguousarray(
        np.asarray(a, np.float32) * scale).astype(ml_dtypes.float8_e4m3)


def _colmaj(v):
    return np.asarray(v, np.float32).reshape(-1, 128).T


def _sbufpack(w):
    """[D_in, D_out] -> [128, (D_in/128)*D_out] partition-contiguous."""
    din, dout = w.shape
    return w.reshape(din // 128, 128, dout).transpose(1, 0, 2).reshape(128, -1)


def _pack_shared(Ws_gate, bs_gate, Ws_up, bs_up, Ws_down, bs_down,
                 Wr_gate, br_gate, Wr_up, br_up, Wr_down, br_down):
    ws = np.empty((128, 2 * 3 * WSZ), np.float32)
    we = np.empty((128, E * 3 * WSZ), np.float32)
    gbt = np.empty((128, NU, NH), np.float32)
    ubt = np.empty((128, NU, NH), np.float32)
    for u in range(2):
        h0 = slice(u * HU, (u + 1) * HU)
        base = u * 3 * WSZ
        ws[:, base:base + WSZ] = _sbufpack(np.asarray(Ws_gate, np.float32)[:, h0])
        ws[:, base + WSZ:base + 2 * WSZ] = _sbufpack(
            np.asarray(Ws_up, np.float32)[:, h0])
        ws[:, base + 2 * WSZ:base + 3 * WSZ] = _sbufpack(
            np.asarray(Ws_down, np.float32)[h0, :])
        gbt[:, u, :] = _colmaj(bs_gate[h0])
        ubt[:, u, :] = _colmaj(bs_up[h0])
    for e in range(E):
        base = e * 3 * WSZ
        we[:, base:base + WSZ] = _sbufpack(np.asarray(Wr_gate, np.float32)[e])
        we[:, base + WSZ:base + 2 * WSZ] = _sbufpack(
            np.asarray(Wr_up, np.float32)[e])
        we[:, base + 2 * WSZ:base + 3 * WSZ] = _sbufpack(
            np.asarray(Wr_down, np.float32)[e])
        gbt[:, 2 + e, :] = _colmaj(br_gate[e])
        ubt[:, 2 + e, :] = _colmaj(br_up[e]) * SXW
    return _bf16(ws), _fp8(we, SWQ), gbt, ubt


def _run(inputs, trace=False):
    x = np.asarray(inputs["x"], np.float32)
    rweights = np.asarray(inputs["routing_weights"], np.float32)
    wsb, we8, gbt, ubt = _pack_shared(
        np.asarray(inputs["Ws_gate"], np.float32), inputs["bs_gate"],
        np.asarray(inputs["Ws_up"], np.float32), inputs["bs_up"],
        np.asarray(inputs["Ws_down"], np.float32), inputs["bs_down"],
        np.asarray(inputs["Wr_gate"], np.float32), inputs["br_gate"],
        np.asarray(inputs["Wr_up"], np.float32), inputs["br_up"],
        np.asarray(inputs["Wr_down"], np.float32), inputs["br_down"])
    bs_down = np.asarray(inputs["bs_down"], np.float32)
    br_down = np.asarray(inputs["br_down"], np.float32)
    # down-bias vector per batch: bs_down + sum_e rw[b,e]*br_down[e]
    cfull = bs_down[None, :] + rweights @ br_down       # [B, D]

    in_maps = []
    for i in range(NCORES):
        xT = x[BL * i:BL * (i + 1)].reshape(T, D).T     # [D, T]
        # pack x to [128, NK, T]: partition p, block k, token t = xT[k*128+p, t]
        xP = xT.reshape(NK, 128, T).transpose(1, 0, 2)
        rwtab = np.empty((128, NU, NTT), np.float32)
        rwtab[:, :2, :] = SDC
        cvtab = np.empty((128, ND, NTT), np.float32)
        for t in range(NTT):
            bg = BL * i + t // (K // TT)
            for e in range(E):
                rwtab[:, 2 + e, t] = rweights[bg, e] * (C / SXW)
            cvtab[:, :, t] = cfull[bg].reshape(ND, 128).T
        in_maps.append({"xTb": _bf16(xP), "xT8": _fp8(xP, SX),
                        "wsb": wsb, "we8": we8,
                        "gb": gbt, "ub": ubt,
                        "rw": np.ascontiguousarray(rwtab),
                        "cv": np.ascontiguousarray(cvtab)})

    fused = (not np.any(np.asarray(inputs["bs_up"], np.float32))
             and not np.any(np.asarray(inputs["br_up"], np.float32)))
    res = bass_utils.run_bass_kernel_spmd(_get_nc(fused), in_maps,
                                          core_ids=list(range(NCORES)),
                                          trace=trace)
    out = np.empty((B, K, D), np.float32)
    for i in range(NCORES):
        out[BL * i:BL * (i + 1)] = res.results[i]["outT"].T.reshape(BL, K, D)
    return out, res


def kernel(**inputs) -> np.ndarray:
    out, _ = _run(inputs, trace=False)
    return out
